# revision 5
# baseline (speedup 1.0000x reference)
"""Trainium2 Bass kernel for nn_G_HGNN_layer_38448547234609.

HGNN layer: knn-hypergraph (top-11 of 8192 nodes) + static local hyperedges,
G = Dv^-1/2 H De^-1 H^T Dv^-1/2 message passing, linear -> G@y -> BN -> relu
-> residual.  Never materializes G.

Sharding: core c owns sample c (1024 nodes = 8 row-tiles of 128).

Key structure (per core):
 - distances via 2 bf16 matmuls (bf16x2 split: hi/lo of x; sq split in 3 bf16
   rows) at 1 cyc/row instead of fp32's 4 -> exact-enough top-11 (0 flips vs
   fp32 reference on this data).
 - d row tile [128, 8192] f32 staged in an SBUF ring; DVE max8 -> exact
   top-11 threshold; is_ge -> 0/1 masks in fp8e4 (split DVE/gpsimd).
 - u^T = Hknn^T @ [m|1] flipped: lhsT = m_aug (stationary), mask streamed ->
   [65, 8192] in 2 half-sweeps; drained fp16; TWO AllReduces (halves) so the
   second overlaps compute.
 - z^T[c,i] = sum_j v[j,c] maskT[j,i]: mask tiles PE-transposed (bit-exact),
   v from reduced u (per-chunk transpose + recip), local hyperedges via
   static H_local^T matmuls; BN stats + epilogue in z^T space; final PE
   transposes -> out.
"""

import numpy as np
import ml_dtypes

import concourse.bass as bass
import concourse.bacc as bacc
import concourse.mybir as mybir
import concourse.tile as tile
from concourse import bass_utils

AF = mybir.ActivationFunctionType
ALU = mybir.AluOpType
F32 = mybir.dt.float32
F16 = mybir.dt.float16
BF16 = mybir.dt.bfloat16
F8 = mybir.dt.float8e4

NODE, K, KER, STR = 32, 10, 5, 2
B, C = 8, 64
N = NODE * NODE            # 1024 nodes/sample
BN = B * N                 # 8192
OUT_ = (NODE - KER) // STR + 1
E = OUT_ * OUT_            # 196 local hyperedges/sample
NCORE = 8
NT = 8                     # 128-row tiles per core
JC = 64                    # 128-col j-chunks
NCK = 16                   # 512-col chunks per row tile
BN_EPS = 1e-5
BIG = 1e30
NRING = 18                 # d-chunk ring slots
NROLL = 10                 # maskT rolling chunks

_CACHE = {}


def _local_incidence():
    idx = np.arange(N).reshape(NODE, NODE)
    H_local = np.zeros((N, E), np.float32)
    e = 0
    for i in range(0, NODE - KER + 1, STR):
        for j in range(0, NODE - KER + 1, STR):
            H_local[idx[i:i + KER, j:j + KER].ravel(), e] = 1.0
            e += 1
    return H_local


def _build():
    nc = bacc.Bacc(num_devices=NCORE)

    r1 = nc.dram_tensor("r1", [67, BN], BF16, kind="ExternalInput")
    r2 = nc.dram_tensor("r2", [128, BN], BF16, kind="ExternalInput")
    a1 = nc.dram_tensor("a1", [67, N], BF16, kind="ExternalInput")
    a2 = nc.dram_tensor("a2", [128, N], BF16, kind="ExternalInput")
    wbb = nc.dram_tensor("wbb", [65, C], BF16, kind="ExternalInput")
    dv2t = nc.dram_tensor("dv2t", [128, NT], F32, kind="ExternalInput")
    dv2r = nc.dram_tensor("dv2r", [1, N], F32, kind="ExternalInput")
    hloc = nc.dram_tensor("hloc", [128, NT * E], BF16, kind="ExternalInput")
    hlt = nc.dram_tensor("hlt", [98, 2 * N], BF16, kind="ExternalInput")
    id8 = nc.dram_tensor("id8", [128, 128], F8, kind="ExternalInput")
    id16 = nc.dram_tensor("id16", [128, 128], F16, kind="ExternalInput")
    id32 = nc.dram_tensor("id32", [128, 128], F32, kind="ExternalInput")
    gb = nc.dram_tensor("gb", [C, 2], F32, kind="ExternalInput")
    xtr = nc.dram_tensor("xtr", [C, N], F32, kind="ExternalInput")
    out = nc.dram_tensor("out", [N, C], F32, kind="ExternalOutput")

    with tile.TileContext(nc) as tc:
        with (
            tc.tile_pool(name="const", bufs=1) as cp,
            tc.tile_pool(name="dring", bufs=NRING) as dp,
            tc.tile_pool(name="cands", bufs=2) as cnp,
            tc.tile_pool(name="roll", bufs=NROLL) as rp,
            tc.tile_pool(name="small", bufs=4) as sp,
            tc.tile_pool(name="persist", bufs=1) as pp,
            tc.tile_pool(name="dram", bufs=1, space="DRAM") as dr,
        ):
            # ---- const loads ----
            r1_sb = cp.tile([67, BN], BF16, tag="r1")
            nc.sync.dma_start(r1_sb[:], r1[:])
            r2_sb = cp.tile([128, BN], BF16, tag="r2")
            nc.sync.dma_start(r2_sb[:], r2[:])
            a1_sb = cp.tile([67, N], BF16, tag="a1")
            nc.sync.dma_start(a1_sb[:], a1[:])
            a2_sb = cp.tile([128, N], BF16, tag="a2")
            nc.sync.dma_start(a2_sb[:], a2[:])
            wb_sb = cp.tile([65, C], BF16, tag="wbb")
            nc.sync.dma_start(wb_sb[:], wbb[:])
            dv2_sb = cp.tile([128, NT], F32, tag="dv2")
            nc.sync.dma_start(dv2_sb[:], dv2t[:])
            dv2r_sb = cp.tile([1, N], F32, tag="dv2r")
            nc.sync.dma_start(dv2r_sb[:], dv2r[:])
            hloc_sb = cp.tile([128, NT * E], BF16, tag="hloc")
            nc.sync.dma_start(hloc_sb[:], hloc[:])
            hlt_sb = cp.tile([98, 2 * N], BF16, tag="hlt")
            nc.sync.dma_start(hlt_sb[:], hlt[:])
            id8_sb = cp.tile([128, 128], F8, tag="id8")
            nc.sync.dma_start(id8_sb[:], id8[:])
            id16_sb = cp.tile([128, 128], F16, tag="id16")
            nc.sync.dma_start(id16_sb[:], id16[:])
            id32_sb = cp.tile([128, 128], F32, tag="id32")
            nc.sync.dma_start(id32_sb[:], id32[:])
            gb_sb = cp.tile([C, 2], F32, tag="gb")
            nc.sync.dma_start(gb_sb[:], gb[:])
            xtr_sb = cp.tile([C, N], F32, tag="xtr")
            nc.sync.dma_start(xtr_sb[:], xtr[:])

            m_aug = pp.tile([128, NT * 65], BF16, tag="maug")
            masks = [pp.tile([128, BN], F8, tag=f"mask{i}", name=f"mask{i}")
                     for i in range(NT)]
            vloc_sb = pp.tile([98, 2 * C], BF16, tag="vloc")
            dv2b = pp.tile([C, N], F32, tag="dv2b")
            zs_sb = pp.tile([C, N], F32, tag="zs")

            nc.gpsimd.partition_broadcast(dv2b[:], dv2r_sb[:])

            # ---- P0: y = hi^T Wb + b ; m_aug = dv2*y (bf16) + ones col;
            #          t_loc accumulation ----
            with (
                tc.tile_pool(name="py", bufs=2, space="PSUM") as pyp,
                tc.tile_pool(name="ptl", bufs=2, space="PSUM") as ptlp,
            ):
                for it in range(NT):
                    y_ps = pyp.tile([128, C], F32, tag="y")
                    nc.tensor.matmul(y_ps[:], lhsT=a1_sb[0:65, it * 128:(it + 1) * 128],
                                     rhs=wb_sb[:], start=True, stop=True)
                    nc.scalar.activation(m_aug[:, it * 65:it * 65 + C], y_ps[:],
                                         AF.Copy, bias=0.0, scale=dv2_sb[:, it:it + 1])
                    nc.vector.memset(m_aug[:, it * 65 + C:it * 65 + 65], 1.0)
                tl_ps = [ptlp.tile([98, 65], F32, tag=f"tl{ec}", name=f"tl{ec}")
                         for ec in range(2)]
                for it in range(NT):
                    for ec in range(2):
                        nc.tensor.matmul(tl_ps[ec][:],
                                         lhsT=hloc_sb[:, it * E + ec * 98:it * E + ec * 98 + 98],
                                         rhs=m_aug[:, it * 65:(it + 1) * 65],
                                         start=(it == 0), stop=(it == NT - 1))
                for ec in range(2):
                    nc.scalar.activation(vloc_sb[:, ec * C:(ec + 1) * C],
                                         tl_ps[ec][:, 0:C],
                                         AF.Copy, bias=0.0, scale=1.0 / 25.0)

            # ---- P1: distances, top-11 threshold, masks (fp8) ----
            with tc.tile_pool(name="pd", bufs=3, space="PSUM") as pdp:
                for it in range(NT):
                    cand = cnp.tile([128, 128], F32, tag="cand")
                    dslots = []
                    for ck in range(NCK):
                        d_ps = pdp.tile([128, 512], F32, tag="dch")
                        nc.tensor.matmul(d_ps[:],
                                         lhsT=a1_sb[:, it * 128:(it + 1) * 128],
                                         rhs=r1_sb[:, ck * 512:(ck + 1) * 512],
                                         start=True, stop=False)
                        nc.tensor.matmul(d_ps[:],
                                         lhsT=a2_sb[:, it * 128:(it + 1) * 128],
                                         rhs=r2_sb[:, ck * 512:(ck + 1) * 512],
                                         start=False, stop=True)
                        dch = dp.tile([128, 512], F32, tag="dring")
                        nc.scalar.copy(dch[:], d_ps[:])
                        nc.vector.max(cand[:, ck * 8:(ck + 1) * 8], dch[:])
                        dslots.append(dch)
                    c8a = sp.tile([128, 8], F32, tag="c8a")
                    nc.vector.max(c8a[:], cand[:])
                    nc.vector.match_replace(cand[:], c8a[:], cand[:], -BIG)
                    c8b = sp.tile([128, 8], F32, tag="c8b")
                    nc.vector.max(c8b[:], cand[:])
                    # threshold = 11th largest (c8b[2]); is_ge keeps exactly 11
                    for ck in range(NCK):
                        eng = nc.gpsimd if (ck % 8) >= 5 else nc.vector
                        eng.tensor_scalar(masks[it][:, ck * 512:(ck + 1) * 512],
                                          dslots[ck][:], c8b[:, 2:3], None,
                                          ALU.is_ge)

            # ---- P2: u^T half-sweeps + split AllReduce (fp16) ----
            cc_in = [dr.tile([65, BN // 2], F16, tag=f"ccin{h}", name=f"ccin{h}")
                     for h in range(2)]
            cc_out = [dr.tile([65, BN // 2], F16, tag=f"ccout{h}",
                              name=f"ccout{h}", addr_space="Shared")
                      for h in range(2)]
            for h in range(2):
                with tc.tile_pool(name=f"pu{h}", bufs=1, space="PSUM") as pup:
                    u_ps = pup.tile([65, BN // 2], F32, tag="u")
                    for it in range(NT):
                        for jb in range(8):
                            nc.tensor.matmul(
                                u_ps[:, jb * 512:(jb + 1) * 512],
                                lhsT=m_aug[:, it * 65:(it + 1) * 65],
                                rhs=masks[it][:, h * 4096 + jb * 512:
                                              h * 4096 + (jb + 1) * 512],
                                start=(it == 0), stop=(it == NT - 1),
                                skip_group_check=True)
                    for q in range(4):
                        ud = sp.tile([65, 1024], F16, tag="ud")
                        nc.scalar.copy(ud[:], u_ps[:, q * 1024:(q + 1) * 1024])
                        nc.sync.dma_start(cc_in[h][:, q * 1024:(q + 1) * 1024],
                                          ud[:])
                nc.gpsimd.collective_compute(
                    "AllReduce", ALU.add, replica_groups=[list(range(NCORE))],
                    ins=[cc_in[h].opt()], outs=[cc_out[h].opt()])

            # ---- P3/P4: mask transposes, v, z^T accumulation ----
            with (
                tc.tile_pool(name="ptp", bufs=3, space="PSUM") as ptp,
                tc.tile_pool(name="pup2", bufs=3, space="PSUM") as pup2,
                tc.tile_pool(name="pz", bufs=1, space="PSUM") as pzp,
            ):
                zt_ps = pzp.tile([C, N], F32, tag="zt")
                for jc in range(JC):
                    mt = rp.tile([128, 1024], BF16, tag="mt")
                    for it in range(NT):
                        # fp8 transpose requires output element step of 2
                        t_ps = ptp.tile([128, 256], F8, tag="tp")
                        nc.tensor.transpose(t_ps[:, 0:256:2],
                                            masks[it][:, jc * 128:(jc + 1) * 128],
                                            id8_sb[:])
                        nc.scalar.copy(mt[:, it * 128:(it + 1) * 128],
                                       t_ps[:, 0:256:2])
                    h = jc // 32
                    uch = sp.tile([65, 128], F16, tag="uch")
                    nc.sync.dma_start(
                        uch[:], cc_out[h][:, (jc - h * 32) * 128:
                                          (jc - h * 32 + 1) * 128])
                    ut_ps = pup2.tile([128, 65], F16, tag="utp")
                    nc.tensor.transpose(ut_ps[:], uch[:], id16_sb[0:65, 0:65])
                    ut = sp.tile([128, 65], F32, tag="ut")
                    nc.scalar.copy(ut[:], ut_ps[:])
                    rec = sp.tile([128, 1], F32, tag="rec")
                    nc.vector.reciprocal(rec[:], ut[:, 64:65])
                    v = sp.tile([128, C], BF16, tag="v")
                    nc.vector.tensor_scalar(v[:], ut[:, 0:C], rec[:, 0:1],
                                            None, ALU.mult)
                    for half in range(2):
                        nc.tensor.matmul(zt_ps[:, half * 512:(half + 1) * 512],
                                         lhsT=v[:],
                                         rhs=mt[:, half * 512:(half + 1) * 512],
                                         start=(jc == 0), stop=False,
                                         skip_group_check=True)
                # local hyperedge contribution
                for ec in range(2):
                    for half in range(2):
                        nc.tensor.matmul(zt_ps[:, half * 512:(half + 1) * 512],
                                         lhsT=vloc_sb[:, ec * C:(ec + 1) * C],
                                         rhs=hlt_sb[:, ec * N + half * 512:
                                                    ec * N + (half + 1) * 512],
                                         start=False,
                                         stop=(ec == 1),
                                         skip_group_check=True)
                # drain z^T, scale by dv2 along free dim (in place)
                nc.scalar.copy(zs_sb[:], zt_ps[:])
            nc.vector.tensor_tensor(zs_sb[:], zs_sb[:], dv2b[:], ALU.mult)

            # ---- P5: BN stats + AllReduce + coefficients ----
            ro = pp.tile([C, N], F32, tag="ro")
            stt = sp.tile([C, 2], F32, tag="stt")
            nc.vector.tensor_reduce(stt[:, 0:1], zs_sb[:],
                                    mybir.AxisListType.X, ALU.add)
            nc.vector.tensor_tensor(ro[:], zs_sb[:], zs_sb[:], ALU.mult)
            nc.vector.tensor_reduce(stt[:, 1:2], ro[:],
                                    mybir.AxisListType.X, ALU.add)
            st_in = dr.tile([C, 2], F32, tag="stin")
            st_out = dr.tile([C, 2], F32, tag="stout", addr_space="Shared")
            nc.sync.dma_start(st_in[:], stt[:])
            nc.gpsimd.collective_compute(
                "AllReduce", ALU.add, replica_groups=[list(range(NCORE))],
                ins=[st_in.opt()], outs=[st_out.opt()])
            stg = sp.tile([C, 2], F32, tag="stg")
            nc.sync.dma_start(stg[:], st_out[:])

            mu = sp.tile([C, 1], F32, tag="mu")
            nc.vector.tensor_scalar(mu[:], stg[:, 0:1], 1.0 / BN, None, ALU.mult)
            ex2 = sp.tile([C, 1], F32, tag="ex2")
            nc.vector.tensor_scalar(ex2[:], stg[:, 1:2], 1.0 / BN, None, ALU.mult)
            musq = sp.tile([C, 1], F32, tag="musq")
            nc.vector.tensor_tensor(musq[:], mu[:], mu[:], ALU.mult)
            var = sp.tile([C, 1], F32, tag="var")
            nc.vector.tensor_tensor(var[:], ex2[:], musq[:], ALU.subtract)
            eps_t = sp.tile([C, 1], F32, tag="eps")
            nc.vector.memset(eps_t[:], BN_EPS)
            sd = sp.tile([C, 1], F32, tag="sd")
            nc.scalar.activation(sd[:], var[:], AF.Sqrt, bias=eps_t[:, 0:1], scale=1.0)
            inv = sp.tile([C, 1], F32, tag="inv")
            nc.vector.reciprocal(inv[:], sd[:])
            s_col = sp.tile([C, 1], F32, tag="scol")
            nc.vector.tensor_tensor(s_col[:], gb_sb[:, 0:1], inv[:], ALU.mult)
            ms = sp.tile([C, 1], F32, tag="ms")
            nc.vector.tensor_tensor(ms[:], mu[:], s_col[:], ALU.mult)
            t_col = sp.tile([C, 1], F32, tag="tcol")
            nc.vector.tensor_tensor(t_col[:], gb_sb[:, 1:2], ms[:], ALU.subtract)

            # ---- P6: relu + residual + transpose out ----
            nc.scalar.activation(ro[:], zs_sb[:], AF.Relu,
                                 bias=t_col[:, 0:1], scale=s_col[:, 0:1])
            nc.vector.tensor_tensor(ro[:], ro[:], xtr_sb[:], ALU.add)
            with tc.tile_pool(name="po", bufs=2, space="PSUM") as pop:
                for it in range(NT):
                    o_ps = pop.tile([128, C], F32, tag="ops")
                    nc.tensor.transpose(o_ps[:], ro[:, it * 128:(it + 1) * 128],
                                        id32_sb[0:C, 0:C])
                    ot = sp.tile([128, C], F32, tag="ot")
                    nc.scalar.copy(ot[:], o_ps[:])
                    nc.sync.dma_start(out[it * 128:(it + 1) * 128, :], ot[:])

    nc.compile()
    return nc


def _host_inputs(x, W_conv, b_conv, gamma, beta):
    bf = ml_dtypes.bfloat16
    f8 = ml_dtypes.float8_e4m3
    xm = np.ascontiguousarray(x.reshape(BN, C).astype(np.float32))
    xT = np.ascontiguousarray(xm.T)                       # [64, 8192]
    hi32 = xT.astype(bf).astype(np.float32)
    lo32 = (xT - hi32).astype(bf).astype(np.float32)
    sq = (xm * xm).sum(1).astype(np.float32)
    sqh = sq.astype(bf).astype(np.float32)
    sql = (sq - sqh).astype(bf).astype(np.float32)
    sqll = (sq - sqh - sql).astype(bf).astype(np.float32)

    r1 = np.concatenate([2.0 * hi32, -sqh[None, :], -sql[None, :],
                         -sqll[None, :]], 0).astype(bf)    # [67, 8192]
    r2 = np.concatenate([2.0 * lo32, 2.0 * hi32], 0).astype(bf)  # [128, 8192]

    wbm = np.concatenate([W_conv.T.astype(np.float32),
                          b_conv[None, :].astype(np.float32)], 0).astype(bf)

    H_local = _local_incidence()
    cover = H_local.sum(1)
    dv2 = ((K + 1 + cover) ** -0.5).astype(np.float32)     # [1024]
    dv2t = dv2.reshape(NT, 128).T.copy()                   # [128, NT]

    hloc = np.zeros((128, NT * E), np.float32)
    for it in range(NT):
        hloc[:, it * E:(it + 1) * E] = H_local[it * 128:(it + 1) * 128, :]
    hlt = np.zeros((98, 2 * N), np.float32)
    for ec in range(2):
        hlt[:, ec * N:(ec + 1) * N] = H_local[:, ec * 98:(ec + 1) * 98].T

    ident = np.eye(128, dtype=np.float32)
    gb = np.stack([gamma.astype(np.float32), beta.astype(np.float32)], 1)

    common = {
        "r1": np.ascontiguousarray(r1),
        "r2": np.ascontiguousarray(r2),
        "wbb": wbm,
        "dv2t": dv2t,
        "dv2r": dv2[None, :].copy(),
        "hloc": hloc.astype(bf),
        "hlt": hlt.astype(bf),
        "id8": ident.astype(f8),
        "id16": ident.astype(np.float16),
        "id32": ident,
        "gb": gb,
    }
    in_maps = []
    for c in range(NCORE):
        m = dict(common)
        m["a1"] = np.ascontiguousarray(np.concatenate(
            [hi32[:, c * N:(c + 1) * N], np.ones((3, N), np.float32)], 0
        ).astype(bf))
        m["a2"] = np.ascontiguousarray(np.concatenate(
            [hi32[:, c * N:(c + 1) * N], lo32[:, c * N:(c + 1) * N]], 0
        ).astype(bf))
        m["xtr"] = np.ascontiguousarray(xT[:, c * N:(c + 1) * N])
        in_maps.append(m)
    return in_maps


def _get_nc():
    if "nc" not in _CACHE:
        _CACHE["nc"] = _build()
    return _CACHE["nc"]


def run_spmd(inputs, **kw):
    nc = _get_nc()
    in_maps = _host_inputs(inputs["x"], inputs["W_conv"], inputs["b_conv"],
                           inputs["gamma"], inputs["beta"])
    return bass_utils.run_bass_kernel_spmd(nc, in_maps, core_ids=list(range(NCORE)), **kw)


def kernel(**inputs):
    res = run_spmd(inputs)
    out = np.stack([res.results[c]["out"] for c in range(NCORE)], 0)
    return out.reshape(B, N, C).astype(np.float32)


# revision 7
# speedup vs baseline: 1.3162x; 1.3162x over previous
"""Trainium2 Bass kernel for nn_G_HGNN_layer_38448547234609.

HGNN layer: knn-hypergraph (top-11 of 8192 nodes) + static local hyperedges,
G = Dv^-1/2 H De^-1 H^T Dv^-1/2 message passing, linear -> G@y -> BN -> relu
-> residual.  Never materializes G.

Sharding: core c owns sample c (1024 nodes = 8 row-tiles of 128).

Structure (per core):
 - distances d[i,j] = 2<x_i,x_j> - sq_j via 2 bf16 matmuls (bf16x2 split of x,
   sq split into 3 bf16 rows) at 1 cyc/row; exact vs fp32 top-11 on this data.
 - d row chunks drained to an SBUF ring; DVE max8 -> exact top-11 midpoint
   threshold; ACT Sign -> {-1,+1} masks bf16 (single pass).  Masks for row
   tiles 0-2 stay in SBUF, 3-7 spill to DRAM.
 - u^T = Hknn^T @ [m|1]: lhsT = m_aug stationary, +-1 mask streamed; the 0/1
   correction u = 0.5*(u~ + S) is applied per-core before the fp16 cast
   (S = column sums of m_aug, local ones-matmul), so the two half AllReduces
   (fp16) directly sum the corrected partials.
 - z^T[c,i] = sum_j v'[j,c] maskT[j,i] with v' = 0.5 * t/De; maskT tiles come
   from the DMA XBAR transpose engine (bit-exact, no PE/ACT cost); the +-1
   correction is a rank-1 fp16 matmul T'[c] x ones[i]; local hyperedges via
   static H_local^T matmuls; BN stats + epilogue in z^T space; final PE
   transposes -> out.
"""

import numpy as np
import ml_dtypes

import concourse.bass as bass
import concourse.bacc as bacc
import concourse.mybir as mybir
import concourse.tile as tile
from concourse import bass_utils

AF = mybir.ActivationFunctionType
ALU = mybir.AluOpType
F32 = mybir.dt.float32
F16 = mybir.dt.float16
BF16 = mybir.dt.bfloat16

NODE, K, KER, STR = 32, 10, 5, 2
B, C = 8, 64
N = NODE * NODE            # 1024 nodes/sample
BN = B * N                 # 8192
OUT_ = (NODE - KER) // STR + 1
E = OUT_ * OUT_            # 196 local hyperedges/sample
NCORE = 8
NT = 8                     # 128-row tiles per core
JC = 64                    # 128-col j-chunks
NCK = 16                   # 512-col chunks per row tile
BN_EPS = 1e-5
BIG = 1e30
NRING = 18                 # d-chunk ring slots
NSBM = 3                   # masks kept in SBUF (rest spilled to DRAM)
GJC = 16                   # jc per maskT transpose group
NGRP = JC // GJC

_CACHE = {}


def _local_incidence():
    idx = np.arange(N).reshape(NODE, NODE)
    H_local = np.zeros((N, E), np.float32)
    e = 0
    for i in range(0, NODE - KER + 1, STR):
        for j in range(0, NODE - KER + 1, STR):
            H_local[idx[i:i + KER, j:j + KER].ravel(), e] = 1.0
            e += 1
    return H_local


def _build():
    nc = bacc.Bacc(num_devices=NCORE)

    r1 = nc.dram_tensor("r1", [67, BN], BF16, kind="ExternalInput")
    r2 = nc.dram_tensor("r2", [128, BN], BF16, kind="ExternalInput")
    a1 = nc.dram_tensor("a1", [67, N], BF16, kind="ExternalInput")
    a2 = nc.dram_tensor("a2", [128, N], BF16, kind="ExternalInput")
    wbb = nc.dram_tensor("wbb", [65, C], BF16, kind="ExternalInput")
    dv2t = nc.dram_tensor("dv2t", [128, NT], F32, kind="ExternalInput")
    dv2r = nc.dram_tensor("dv2r", [1, N], F32, kind="ExternalInput")
    hloc = nc.dram_tensor("hloc", [128, NT * E], BF16, kind="ExternalInput")
    hlt = nc.dram_tensor("hlt", [98, 2 * N], BF16, kind="ExternalInput")
    id16 = nc.dram_tensor("id16", [128, 128], F16, kind="ExternalInput")
    id32 = nc.dram_tensor("id32", [128, 128], F32, kind="ExternalInput")
    gb = nc.dram_tensor("gb", [C, 2], F32, kind="ExternalInput")
    xtr = nc.dram_tensor("xtr", [C, N], F32, kind="ExternalInput")
    out = nc.dram_tensor("out", [N, C], F32, kind="ExternalOutput")

    with tile.TileContext(nc) as tc:
        with (
            tc.tile_pool(name="const", bufs=1) as cp,
            tc.tile_pool(name="cands", bufs=2) as cnp,
            tc.tile_pool(name="small", bufs=4) as sp,
            tc.tile_pool(name="persist", bufs=1) as pp,
            tc.tile_pool(name="dram", bufs=1, space="DRAM") as dr,
        ):
            # ---- const loads ----
            r1_sb = cp.tile([67, BN], BF16, tag="r1")
            nc.sync.dma_start(r1_sb[:], r1[:])
            r2_sb = cp.tile([128, BN], BF16, tag="r2")
            nc.sync.dma_start(r2_sb[:], r2[:])
            a1_sb = cp.tile([67, N], BF16, tag="a1")
            nc.sync.dma_start(a1_sb[:], a1[:])
            a2_sb = cp.tile([128, N], BF16, tag="a2")
            nc.sync.dma_start(a2_sb[:], a2[:])
            wb_sb = cp.tile([65, C], BF16, tag="wbb")
            nc.sync.dma_start(wb_sb[:], wbb[:])
            dv2_sb = cp.tile([128, NT], F32, tag="dv2")
            nc.sync.dma_start(dv2_sb[:], dv2t[:])
            dv2r_sb = cp.tile([1, N], F32, tag="dv2r")
            nc.sync.dma_start(dv2r_sb[:], dv2r[:])
            hloc_sb = cp.tile([128, NT * E], BF16, tag="hloc")
            nc.sync.dma_start(hloc_sb[:], hloc[:])
            hlt_sb = cp.tile([98, 2 * N], BF16, tag="hlt")
            nc.sync.dma_start(hlt_sb[:], hlt[:])
            id16_sb = cp.tile([128, 128], F16, tag="id16")
            nc.sync.dma_start(id16_sb[:], id16[:])
            id32_sb = cp.tile([128, 128], F32, tag="id32")
            nc.sync.dma_start(id32_sb[:], id32[:])
            gb_sb = cp.tile([C, 2], F32, tag="gb")
            nc.sync.dma_start(gb_sb[:], gb[:])
            xtr_sb = cp.tile([C, N], F32, tag="xtr")
            nc.sync.dma_start(xtr_sb[:], xtr[:])

            m_aug = pp.tile([128, NT * 65], BF16, tag="maug")
            masks_sb = [pp.tile([128, BN], BF16, tag=f"mask{i}", name=f"mask{i}")
                        for i in range(NSBM)]
            masks_dr = [dr.tile([128, BN], BF16, tag=f"maskd{i}", name=f"maskd{i}")
                        for i in range(NSBM, NT)]
            vloc_sb = pp.tile([98, 2 * C], BF16, tag="vloc")
            dv2b = pp.tile([C, N], F32, tag="dv2b")
            zs_sb = pp.tile([C, N], F32, tag="zs")
            s_col = pp.tile([65, 1], F32, tag="scol")
            ones128 = pp.tile([128, 1], BF16, tag="ones128")
            nc.vector.memset(ones128[:], 1.0)
            ones512 = pp.tile([1, 512], F16, tag="ones512")
            nc.vector.memset(ones512[:], 1.0)

            nc.gpsimd.partition_broadcast(dv2b[:], dv2r_sb[:])

            # ---- P0: y = hi^T Wb + b ; m_aug = dv2*y (bf16) + ones col;
            #          t_loc accumulation; S = column sums of m_aug ----
            with (
                tc.tile_pool(name="py", bufs=2, space="PSUM") as pyp,
                tc.tile_pool(name="ptl", bufs=2, space="PSUM") as ptlp,
            ):
                for it in range(NT):
                    y_ps = pyp.tile([128, C], F32, tag="y")
                    nc.tensor.matmul(y_ps[:], lhsT=a1_sb[0:65, it * 128:(it + 1) * 128],
                                     rhs=wb_sb[:], start=True, stop=True)
                    nc.scalar.activation(m_aug[:, it * 65:it * 65 + C], y_ps[:],
                                         AF.Copy, bias=0.0, scale=dv2_sb[:, it:it + 1])
                    nc.vector.memset(m_aug[:, it * 65 + C:it * 65 + 65], 1.0)
                tl_ps = [ptlp.tile([98, 65], F32, tag=f"tl{ec}", name=f"tl{ec}")
                         for ec in range(2)]
                for it in range(NT):
                    for ec in range(2):
                        nc.tensor.matmul(tl_ps[ec][:],
                                         lhsT=hloc_sb[:, it * E + ec * 98:it * E + ec * 98 + 98],
                                         rhs=m_aug[:, it * 65:(it + 1) * 65],
                                         start=(it == 0), stop=(it == NT - 1))
                for ec in range(2):
                    nc.scalar.activation(vloc_sb[:, ec * C:(ec + 1) * C],
                                         tl_ps[ec][:, 0:C],
                                         AF.Copy, bias=0.0, scale=1.0 / 25.0)
                s_ps = ptlp.tile([65, 1], F32, tag="sps")
                for it in range(NT):
                    nc.tensor.matmul(s_ps[:], lhsT=m_aug[:, it * 65:(it + 1) * 65],
                                     rhs=ones128[:], start=(it == 0),
                                     stop=(it == NT - 1))
                nc.scalar.copy(s_col[:], s_ps[:])

            # ---- P1: distances -> ring; top-11 midpoint; Sign -> +-1 masks ----
            with (
                tc.tile_pool(name="dring", bufs=NRING) as dp,
                tc.tile_pool(name="spill", bufs=2) as spl,
                tc.tile_pool(name="pd", bufs=3, space="PSUM") as pdp,
            ):
                for it in range(NT):
                    if it < NSBM:
                        mk = masks_sb[it]
                    else:
                        mk = spl.tile([128, BN], BF16, tag="mspill")
                    cand = cnp.tile([128, 128], F32, tag="cand")
                    dslots = []
                    for ck in range(NCK):
                        d_ps = pdp.tile([128, 512], F32, tag="dch")
                        nc.tensor.matmul(d_ps[:],
                                         lhsT=a1_sb[:, it * 128:(it + 1) * 128],
                                         rhs=r1_sb[:, ck * 512:(ck + 1) * 512],
                                         start=True, stop=False)
                        nc.tensor.matmul(d_ps[:],
                                         lhsT=a2_sb[:, it * 128:(it + 1) * 128],
                                         rhs=r2_sb[:, ck * 512:(ck + 1) * 512],
                                         start=False, stop=True)
                        dch = dp.tile([128, 512], F32, tag="dring")
                        nc.scalar.copy(dch[:], d_ps[:])
                        nc.vector.max(cand[:, ck * 8:(ck + 1) * 8], dch[:])
                        dslots.append(dch)
                    c8a = sp.tile([128, 8], F32, tag="c8a")
                    nc.vector.max(c8a[:], cand[:])
                    nc.vector.match_replace(cand[:], c8a[:], cand[:], -BIG)
                    c8b = sp.tile([128, 8], F32, tag="c8b")
                    nc.vector.max(c8b[:], cand[:])
                    # negative midpoint between 11th and 12th largest
                    tmn = sp.tile([128, 1], F32, tag="tmn")
                    nc.vector.tensor_tensor(tmn[:], c8b[:, 2:3], c8b[:, 3:4], ALU.add)
                    nc.vector.tensor_scalar(tmn[:], tmn[:], -0.5, None, ALU.mult)
                    for ck in range(NCK):
                        nc.scalar.activation(mk[:, ck * 512:(ck + 1) * 512],
                                             dslots[ck][:], AF.Sign,
                                             bias=tmn[:, 0:1], scale=1.0)
                    if it >= NSBM:
                        nc.sync.dma_start(masks_dr[it - NSBM][:], mk[:])

            # ---- P2: u^T half-sweeps with +-1 fix, split AllReduce (fp16) ----
            cc_in = [dr.tile([65, BN // 2], F16, tag=f"ccin{h}", name=f"ccin{h}")
                     for h in range(2)]
            cc_out = [dr.tile([65, BN // 2], F16, tag=f"ccout{h}",
                              name=f"ccout{h}", addr_space="Shared")
                      for h in range(2)]
            for h in range(2):
                with (
                    tc.tile_pool(name=f"pu{h}", bufs=1, space="PSUM") as pup,
                    tc.tile_pool(name=f"rstr{h}", bufs=3) as rsp,
                ):
                    u_ps = pup.tile([65, BN // 2], F32, tag="u")
                    for it in range(NT):
                        if it < NSBM:
                            mhalf = masks_sb[it][:, h * 4096:(h + 1) * 4096]
                        else:
                            mhalf_t = rsp.tile([128, 4096], BF16, tag="rst")
                            nc.sync.dma_start(
                                mhalf_t[:],
                                masks_dr[it - NSBM][:, h * 4096:(h + 1) * 4096])
                            mhalf = mhalf_t[:]
                        for jb in range(8):
                            nc.tensor.matmul(
                                u_ps[:, jb * 512:(jb + 1) * 512],
                                lhsT=m_aug[:, it * 65:(it + 1) * 65],
                                rhs=mhalf[:, jb * 512:(jb + 1) * 512],
                                start=(it == 0), stop=(it == NT - 1),
                                skip_group_check=True)
                    # u = 0.5*(u~ + S) applied on drain, cast fp16
                    for q in range(4):
                        ud = sp.tile([65, 1024], F16, tag="ud")
                        nc.vector.tensor_scalar(ud[:], u_ps[:, q * 1024:(q + 1) * 1024],
                                                s_col[:, 0:1], 0.5, ALU.add, ALU.mult)
                        nc.sync.dma_start(cc_in[h][:, q * 1024:(q + 1) * 1024],
                                          ud[:])
                nc.gpsimd.collective_compute(
                    "AllReduce", ALU.add, replica_groups=[list(range(NCORE))],
                    ins=[cc_in[h].opt()], outs=[cc_out[h].opt()])

            # ---- P3/P4: maskT via DMA XBAR transpose; v; z^T accumulation ----
            with (
                tc.tile_pool(name="roll", bufs=2) as rp,
                tc.tile_pool(name="pup2", bufs=3, space="PSUM") as pup2,
                tc.tile_pool(name="ptq", bufs=2, space="PSUM") as ptq,
                tc.tile_pool(name="pz", bufs=1, space="PSUM") as pzp,
            ):
                zt_ps = pzp.tile([C, N], F32, tag="zt")
                tq_ps = ptq.tile([1, C], F32, tag="tq")
                for g in range(NGRP):
                    mtg = rp.tile([128, GJC * 1024], BF16, tag="mtg")
                    mtg3 = mtg[:].rearrange("p (a b) -> p a b", b=1024)
                    for it in range(NT):
                        if it < NSBM:
                            src = masks_sb[it][:, g * GJC * 128:(g + 1) * GJC * 128]
                        else:
                            src = masks_dr[it - NSBM][:, g * GJC * 128:(g + 1) * GJC * 128]
                        nc.sync.dma_start_transpose(
                            mtg3[:, :, it * 128:(it + 1) * 128], src)
                    for jl in range(GJC):
                        jc = g * GJC + jl
                        h = jc // 32
                        uch = sp.tile([65, 128], F16, tag="uch")
                        nc.sync.dma_start(
                            uch[:], cc_out[h][:, (jc - h * 32) * 128:
                                              (jc - h * 32 + 1) * 128])
                        ut_ps = pup2.tile([128, 65], F16, tag="utp")
                        nc.tensor.transpose(ut_ps[:], uch[:], id16_sb[0:65, 0:65])
                        ut = sp.tile([128, 65], F32, tag="ut")
                        nc.scalar.copy(ut[:], ut_ps[:])
                        rec = sp.tile([128, 1], F32, tag="rec")
                        nc.vector.reciprocal(rec[:], ut[:, 64:65])
                        # v' = 0.5 * t / De  (bf16)
                        v = sp.tile([128, C], BF16, tag="v")
                        nc.vector.tensor_scalar(v[:], ut[:, 0:C], rec[:, 0:1],
                                                0.5, ALU.mult, ALU.mult)
                        for half in range(2):
                            nc.tensor.matmul(
                                zt_ps[:, half * 512:(half + 1) * 512],
                                lhsT=v[:],
                                rhs=mtg[:, jl * 1024 + half * 512:
                                        jl * 1024 + (half + 1) * 512],
                                start=(jc == 0), stop=False,
                                skip_group_check=True)
                        # T'[c] accumulation: sum_j v'[j, c] -> [1, C]
                        nc.tensor.matmul(tq_ps[:], lhsT=ones128[:, 0:1], rhs=v[:],
                                         start=(jc == 0), stop=(jc == JC - 1),
                                         skip_group_check=True)
                # rank-1 correction + local hyperedges into z^T
                tq16 = sp.tile([1, C], F16, tag="tq16")
                nc.scalar.copy(tq16[:], tq_ps[:])
                for half in range(2):
                    nc.tensor.matmul(zt_ps[:, half * 512:(half + 1) * 512],
                                     lhsT=tq16[:],
                                     rhs=ones512[:],
                                     start=False, stop=False,
                                     skip_group_check=True)
                for ec in range(2):
                    for half in range(2):
                        nc.tensor.matmul(zt_ps[:, half * 512:(half + 1) * 512],
                                         lhsT=vloc_sb[:, ec * C:(ec + 1) * C],
                                         rhs=hlt_sb[:, ec * N + half * 512:
                                                    ec * N + (half + 1) * 512],
                                         start=False,
                                         stop=(ec == 1),
                                         skip_group_check=True)
                # drain z^T, scale by dv2 along free dim (in place)
                nc.scalar.copy(zs_sb[:], zt_ps[:])
            nc.vector.tensor_tensor(zs_sb[:], zs_sb[:], dv2b[:], ALU.mult)

            # ---- P5: BN stats + AllReduce + coefficients ----
            ro = pp.tile([C, N], F32, tag="ro")
            stt = sp.tile([C, 2], F32, tag="stt")
            nc.vector.tensor_reduce(stt[:, 0:1], zs_sb[:],
                                    mybir.AxisListType.X, ALU.add)
            nc.vector.tensor_tensor(ro[:], zs_sb[:], zs_sb[:], ALU.mult)
            nc.vector.tensor_reduce(stt[:, 1:2], ro[:],
                                    mybir.AxisListType.X, ALU.add)
            st_in = dr.tile([C, 2], F32, tag="stin")
            st_out = dr.tile([C, 2], F32, tag="stout", addr_space="Shared")
            nc.sync.dma_start(st_in[:], stt[:])
            nc.gpsimd.collective_compute(
                "AllReduce", ALU.add, replica_groups=[list(range(NCORE))],
                ins=[st_in.opt()], outs=[st_out.opt()])
            stg = sp.tile([C, 2], F32, tag="stg")
            nc.sync.dma_start(stg[:], st_out[:])

            mu = sp.tile([C, 1], F32, tag="mu")
            nc.vector.tensor_scalar(mu[:], stg[:, 0:1], 1.0 / BN, None, ALU.mult)
            ex2 = sp.tile([C, 1], F32, tag="ex2")
            nc.vector.tensor_scalar(ex2[:], stg[:, 1:2], 1.0 / BN, None, ALU.mult)
            musq = sp.tile([C, 1], F32, tag="musq")
            nc.vector.tensor_tensor(musq[:], mu[:], mu[:], ALU.mult)
            var = sp.tile([C, 1], F32, tag="var")
            nc.vector.tensor_tensor(var[:], ex2[:], musq[:], ALU.subtract)
            eps_t = sp.tile([C, 1], F32, tag="eps")
            nc.vector.memset(eps_t[:], BN_EPS)
            sd = sp.tile([C, 1], F32, tag="sd")
            nc.scalar.activation(sd[:], var[:], AF.Sqrt, bias=eps_t[:, 0:1], scale=1.0)
            inv = sp.tile([C, 1], F32, tag="inv")
            nc.vector.reciprocal(inv[:], sd[:])
            sc_col = sp.tile([C, 1], F32, tag="sccol")
            nc.vector.tensor_tensor(sc_col[:], gb_sb[:, 0:1], inv[:], ALU.mult)
            ms = sp.tile([C, 1], F32, tag="ms")
            nc.vector.tensor_tensor(ms[:], mu[:], sc_col[:], ALU.mult)
            t_col = sp.tile([C, 1], F32, tag="tcol")
            nc.vector.tensor_tensor(t_col[:], gb_sb[:, 1:2], ms[:], ALU.subtract)

            # ---- P6: relu + residual + transpose out ----
            nc.scalar.activation(ro[:], zs_sb[:], AF.Relu,
                                 bias=t_col[:, 0:1], scale=sc_col[:, 0:1])
            nc.vector.tensor_tensor(ro[:], ro[:], xtr_sb[:], ALU.add)
            with tc.tile_pool(name="po", bufs=2, space="PSUM") as pop:
                for it in range(NT):
                    o_ps = pop.tile([128, C], F32, tag="ops")
                    nc.tensor.transpose(o_ps[:], ro[:, it * 128:(it + 1) * 128],
                                        id32_sb[0:C, 0:C])
                    ot = sp.tile([128, C], F32, tag="ot")
                    nc.scalar.copy(ot[:], o_ps[:])
                    nc.sync.dma_start(out[it * 128:(it + 1) * 128, :], ot[:])

    nc.compile()
    return nc


def _host_inputs(x, W_conv, b_conv, gamma, beta):
    bf = ml_dtypes.bfloat16
    xm = np.ascontiguousarray(x.reshape(BN, C).astype(np.float32))
    xT = np.ascontiguousarray(xm.T)                       # [64, 8192]
    hi32 = xT.astype(bf).astype(np.float32)
    lo32 = (xT - hi32).astype(bf).astype(np.float32)
    sq = (xm * xm).sum(1).astype(np.float32)
    sqh = sq.astype(bf).astype(np.float32)
    sql = (sq - sqh).astype(bf).astype(np.float32)
    sqll = (sq - sqh - sql).astype(bf).astype(np.float32)

    r1m = np.concatenate([2.0 * hi32, -sqh[None, :], -sql[None, :],
                          -sqll[None, :]], 0).astype(bf)   # [67, 8192]
    r2m = np.concatenate([2.0 * lo32, 2.0 * hi32], 0).astype(bf)  # [128, 8192]

    wbm = np.concatenate([W_conv.T.astype(np.float32),
                          b_conv[None, :].astype(np.float32)], 0).astype(bf)

    H_local = _local_incidence()
    cover = H_local.sum(1)
    dv2 = ((K + 1 + cover) ** -0.5).astype(np.float32)     # [1024]
    dv2t = dv2.reshape(NT, 128).T.copy()                   # [128, NT]

    hlocm = np.zeros((128, NT * E), np.float32)
    for it in range(NT):
        hlocm[:, it * E:(it + 1) * E] = H_local[it * 128:(it + 1) * 128, :]
    hltm = np.zeros((98, 2 * N), np.float32)
    for ec in range(2):
        hltm[:, ec * N:(ec + 1) * N] = H_local[:, ec * 98:(ec + 1) * 98].T

    ident = np.eye(128, dtype=np.float32)
    gbm = np.stack([gamma.astype(np.float32), beta.astype(np.float32)], 1)

    common = {
        "r1": np.ascontiguousarray(r1m),
        "r2": np.ascontiguousarray(r2m),
        "wbb": wbm,
        "dv2t": dv2t,
        "dv2r": dv2[None, :].copy(),
        "hloc": hlocm.astype(bf),
        "hlt": hltm.astype(bf),
        "id16": ident.astype(np.float16),
        "id32": ident,
        "gb": gbm,
    }
    in_maps = []
    for c in range(NCORE):
        m = dict(common)
        m["a1"] = np.ascontiguousarray(np.concatenate(
            [hi32[:, c * N:(c + 1) * N], np.ones((3, N), np.float32)], 0
        ).astype(bf))
        m["a2"] = np.ascontiguousarray(np.concatenate(
            [hi32[:, c * N:(c + 1) * N], lo32[:, c * N:(c + 1) * N]], 0
        ).astype(bf))
        m["xtr"] = np.ascontiguousarray(xT[:, c * N:(c + 1) * N])
        in_maps.append(m)
    return in_maps


def _get_nc():
    if "nc" not in _CACHE:
        _CACHE["nc"] = _build()
    return _CACHE["nc"]


def run_spmd(inputs, **kw):
    nc = _get_nc()
    in_maps = _host_inputs(inputs["x"], inputs["W_conv"], inputs["b_conv"],
                           inputs["gamma"], inputs["beta"])
    return bass_utils.run_bass_kernel_spmd(nc, in_maps, core_ids=list(range(NCORE)), **kw)


def kernel(**inputs):
    res = run_spmd(inputs)
    out = np.stack([res.results[c]["out"] for c in range(NCORE)], 0)
    return out.reshape(B, N, C).astype(np.float32)


# revision 9
# speedup vs baseline: 1.4562x; 1.1064x over previous
"""Trainium2 Bass kernel for nn_G_HGNN_layer_38448547234609.

HGNN layer: knn-hypergraph (top-11 of 8192 nodes) + static local hyperedges,
G = Dv^-1/2 H De^-1 H^T Dv^-1/2 message passing, linear -> G@y -> BN -> relu
-> residual.  Never materializes G.

Sharding: core c owns sample c (1024 nodes = 8 row-tiles of 128).

Structure (per core):
 - distances d[i,j] = 2<x_i,x_j> - sq_j via 2 bf16 matmuls (bf16x2 split of x,
   sq split into 3 bf16 rows) at 1 cyc/row; exact vs fp32 top-11 on this data.
 - d row chunks drained to an SBUF ring; DVE max8 -> exact top-11 midpoint
   threshold; ACT Sign -> {-1,+1} masks bf16 (single pass).  Masks for row
   tiles 0-2 stay in SBUF, 3-7 spill to DRAM.
 - u^T = Hknn^T @ [m|1]: lhsT = m_aug stationary, +-1 mask streamed; the 0/1
   correction u = 0.5*(u~ + S) is applied per-core before the fp16 cast
   (S = column sums of m_aug, local ones-matmul), so the two half AllReduces
   (fp16) directly sum the corrected partials.
 - z^T[c,i] = sum_j v'[j,c] maskT[j,i] with v' = 0.5 * t/De; maskT tiles come
   from the DMA XBAR transpose engine (bit-exact, no PE/ACT cost); the +-1
   correction is a rank-1 fp16 matmul T'[c] x ones[i]; local hyperedges via
   static H_local^T matmuls; BN stats + epilogue in z^T space; final PE
   transposes -> out.
"""

import numpy as np
import ml_dtypes

import concourse.bass as bass
import concourse.bacc as bacc
import concourse.mybir as mybir
import concourse.tile as tile
from concourse import bass_utils

AF = mybir.ActivationFunctionType
ALU = mybir.AluOpType
F32 = mybir.dt.float32
F16 = mybir.dt.float16
BF16 = mybir.dt.bfloat16

NODE, K, KER, STR = 32, 10, 5, 2
B, C = 8, 64
N = NODE * NODE            # 1024 nodes/sample
BN = B * N                 # 8192
OUT_ = (NODE - KER) // STR + 1
E = OUT_ * OUT_            # 196 local hyperedges/sample
NCORE = 8
NT = 8                     # 128-row tiles per core
JC = 64                    # 128-col j-chunks
NCK = 16                   # 512-col chunks per row tile
BN_EPS = 1e-5
BIG = 1e30
NRING = 10                 # d-slot ring ([128,1024] slots)
NSBM = 2                   # masks kept in SBUF (rest spilled to DRAM)
GJC = 16                   # jc per maskT transpose group
NGRP = JC // GJC

_CACHE = {}


def _local_incidence():
    idx = np.arange(N).reshape(NODE, NODE)
    H_local = np.zeros((N, E), np.float32)
    e = 0
    for i in range(0, NODE - KER + 1, STR):
        for j in range(0, NODE - KER + 1, STR):
            H_local[idx[i:i + KER, j:j + KER].ravel(), e] = 1.0
            e += 1
    return H_local


def _build():
    nc = bacc.Bacc(num_devices=NCORE)

    r1 = nc.dram_tensor("r1", [67, BN], BF16, kind="ExternalInput")
    r2 = nc.dram_tensor("r2", [128, BN], BF16, kind="ExternalInput")
    a1 = nc.dram_tensor("a1", [67, N], BF16, kind="ExternalInput")
    a2 = nc.dram_tensor("a2", [128, N], BF16, kind="ExternalInput")
    wbb = nc.dram_tensor("wbb", [65, C], BF16, kind="ExternalInput")
    dv2t = nc.dram_tensor("dv2t", [128, NT], F32, kind="ExternalInput")
    dv2r = nc.dram_tensor("dv2r", [1, N], F32, kind="ExternalInput")
    hloc = nc.dram_tensor("hloc", [128, NT * E], BF16, kind="ExternalInput")
    hlt = nc.dram_tensor("hlt", [98, 2 * N], BF16, kind="ExternalInput")
    id16 = nc.dram_tensor("id16", [128, 128], F16, kind="ExternalInput")
    id32 = nc.dram_tensor("id32", [128, 128], F32, kind="ExternalInput")
    gb = nc.dram_tensor("gb", [C, 2], F32, kind="ExternalInput")
    xtr = nc.dram_tensor("xtr", [C, N], F32, kind="ExternalInput")
    out = nc.dram_tensor("out", [N, C], F32, kind="ExternalOutput")

    with tile.TileContext(nc) as tc:
        with (
            tc.tile_pool(name="const", bufs=1) as cp,
            tc.tile_pool(name="cands", bufs=2) as cnp,
            tc.tile_pool(name="small", bufs=4) as sp,
            tc.tile_pool(name="persist", bufs=1) as pp,
            tc.tile_pool(name="dram", bufs=1, space="DRAM") as dr,
        ):
            # ---- const loads ----
            r1_sb = cp.tile([67, BN], BF16, tag="r1")
            nc.sync.dma_start(r1_sb[:], r1[:])
            r2_sb = cp.tile([128, BN], BF16, tag="r2")
            nc.scalar.dma_start(r2_sb[:], r2[:])
            a1_sb = cp.tile([67, N], BF16, tag="a1")
            nc.gpsimd.dma_start(a1_sb[:], a1[:])
            a2_sb = cp.tile([128, N], BF16, tag="a2")
            nc.gpsimd.dma_start(a2_sb[:], a2[:])
            wb_sb = cp.tile([65, C], BF16, tag="wbb")
            nc.sync.dma_start(wb_sb[:], wbb[:])
            dv2_sb = cp.tile([128, NT], F32, tag="dv2")
            nc.sync.dma_start(dv2_sb[:], dv2t[:])
            dv2r_sb = cp.tile([1, N], F32, tag="dv2r")
            nc.sync.dma_start(dv2r_sb[:], dv2r[:])
            hloc_sb = cp.tile([128, NT * E], BF16, tag="hloc")
            nc.gpsimd.dma_start(hloc_sb[:], hloc[:])
            hlt_sb = cp.tile([98, 2 * N], BF16, tag="hlt")
            nc.scalar.dma_start(hlt_sb[:], hlt[:])
            id16_sb = cp.tile([128, 128], F16, tag="id16")
            nc.sync.dma_start(id16_sb[:], id16[:])
            id32_sb = cp.tile([128, 128], F32, tag="id32")
            nc.sync.dma_start(id32_sb[:], id32[:])
            gb_sb = cp.tile([C, 2], F32, tag="gb")
            nc.sync.dma_start(gb_sb[:], gb[:])
            xtr_sb = cp.tile([C, N], F32, tag="xtr")
            nc.sync.dma_start(xtr_sb[:], xtr[:])

            m_aug = pp.tile([128, NT * 65], BF16, tag="maug")
            masks_sb = [pp.tile([128, BN], BF16, tag=f"mask{i}", name=f"mask{i}")
                        for i in range(NSBM)]
            masks_dr = [dr.tile([128, BN], BF16, tag=f"maskd{i}", name=f"maskd{i}")
                        for i in range(NSBM, NT)]
            vloc_sb = pp.tile([98, 2 * C], BF16, tag="vloc")
            dv2b = pp.tile([C, N], F32, tag="dv2b")
            zs_sb = pp.tile([C, N], F32, tag="zs")
            s_col = pp.tile([65, 1], F32, tag="scol")
            ones128 = pp.tile([128, 1], BF16, tag="ones128")
            nc.vector.memset(ones128[:], 1.0)
            ones512 = pp.tile([1, 512], F16, tag="ones512")
            nc.vector.memset(ones512[:], 1.0)

            nc.gpsimd.partition_broadcast(dv2b[:], dv2r_sb[:])

            # ---- P0: y = hi^T Wb + b ; m_aug = dv2*y (bf16) + ones col;
            #          t_loc accumulation; S = column sums of m_aug ----
            with (
                tc.tile_pool(name="py", bufs=2, space="PSUM") as pyp,
                tc.tile_pool(name="ptl", bufs=2, space="PSUM") as ptlp,
            ):
                for it in range(NT):
                    y_ps = pyp.tile([128, C], F32, tag="y")
                    nc.tensor.matmul(y_ps[:], lhsT=a1_sb[0:65, it * 128:(it + 1) * 128],
                                     rhs=wb_sb[:], start=True, stop=True)
                    nc.scalar.activation(m_aug[:, it * 65:it * 65 + C], y_ps[:],
                                         AF.Copy, bias=0.0, scale=dv2_sb[:, it:it + 1])
                    nc.vector.memset(m_aug[:, it * 65 + C:it * 65 + 65], 1.0)
                tl_ps = [ptlp.tile([98, 65], F32, tag=f"tl{ec}", name=f"tl{ec}")
                         for ec in range(2)]
                for it in range(NT):
                    for ec in range(2):
                        nc.tensor.matmul(tl_ps[ec][:],
                                         lhsT=hloc_sb[:, it * E + ec * 98:it * E + ec * 98 + 98],
                                         rhs=m_aug[:, it * 65:(it + 1) * 65],
                                         start=(it == 0), stop=(it == NT - 1))
                for ec in range(2):
                    nc.scalar.activation(vloc_sb[:, ec * C:(ec + 1) * C],
                                         tl_ps[ec][:, 0:C],
                                         AF.Copy, bias=0.0, scale=1.0 / 25.0)
                s_ps = ptlp.tile([65, 1], F32, tag="sps")
                for it in range(NT):
                    nc.tensor.matmul(s_ps[:], lhsT=m_aug[:, it * 65:(it + 1) * 65],
                                     rhs=ones128[:], start=(it == 0),
                                     stop=(it == NT - 1))
                nc.scalar.copy(s_col[:], s_ps[:])

            # ---- P1: distances -> ring; top-11 midpoint; Sign -> +-1 masks ----
            with (
                tc.tile_pool(name="dring", bufs=NRING) as dp,
                tc.tile_pool(name="spill", bufs=2) as spl,
                tc.tile_pool(name="pd", bufs=3, space="PSUM") as pdp,
            ):
                for it in range(NT):
                    if it < NSBM:
                        mk = masks_sb[it]
                    else:
                        mk = spl.tile([128, BN], BF16, tag="mspill")
                    cand = cnp.tile([128, 128], F32, tag="cand")
                    for sl in range(8):
                        d_ps = pdp.tile([128, 1024], F32, tag="dch")
                        for hh in range(2):
                            ck = sl * 2 + hh
                            nc.tensor.matmul(d_ps[:, hh * 512:(hh + 1) * 512],
                                             lhsT=a1_sb[:, it * 128:(it + 1) * 128],
                                             rhs=r1_sb[:, ck * 512:(ck + 1) * 512],
                                             start=True, stop=False,
                                             skip_group_check=True)
                            nc.tensor.matmul(d_ps[:, hh * 512:(hh + 1) * 512],
                                             lhsT=a2_sb[:, it * 128:(it + 1) * 128],
                                             rhs=r2_sb[:, ck * 512:(ck + 1) * 512],
                                             start=False, stop=True,
                                             skip_group_check=True)
                            nc.vector.max(cand[:, (sl * 2 + hh) * 8:
                                                (sl * 2 + hh + 1) * 8],
                                          d_ps[:, hh * 512:(hh + 1) * 512])
                        dch = dp.tile([128, 1024], F32, tag="dring")
                        eng = nc.scalar if sl < 5 else nc.vector
                        if sl < 5:
                            nc.scalar.copy(dch[:], d_ps[:])
                        else:
                            nc.vector.tensor_scalar(dch[:], d_ps[:], 1.0, None,
                                                    ALU.mult)
                        if it == 0 and sl == 0:
                            dslots = []
                        dslots.append(dch)
                    c8a = sp.tile([128, 8], F32, tag="c8a")
                    nc.vector.max(c8a[:], cand[:])
                    nc.vector.match_replace(cand[:], c8a[:], cand[:], -BIG)
                    c8b = sp.tile([128, 8], F32, tag="c8b")
                    nc.vector.max(c8b[:], cand[:])
                    # negative midpoint between 11th and 12th largest
                    tmn = sp.tile([128, 1], F32, tag="tmn")
                    nc.vector.tensor_tensor(tmn[:], c8b[:, 2:3], c8b[:, 3:4], ALU.add)
                    nc.vector.tensor_scalar(tmn[:], tmn[:], -0.5, None, ALU.mult)
                    for sl in range(8):
                        nc.scalar.activation(mk[:, sl * 1024:(sl + 1) * 1024],
                                             dslots[-8 + sl][:], AF.Sign,
                                             bias=tmn[:, 0:1], scale=1.0)
                    if it >= NSBM:
                        nc.sync.dma_start(masks_dr[it - NSBM][:], mk[:])

            # ---- P2: u^T half-sweeps with +-1 fix, split AllReduce (fp16) ----
            cc_in = [dr.tile([65, BN // 2], F16, tag=f"ccin{h}", name=f"ccin{h}")
                     for h in range(2)]
            cc_out = [dr.tile([65, BN // 2], F16, tag=f"ccout{h}",
                              name=f"ccout{h}", addr_space="Shared")
                      for h in range(2)]
            for h in range(2):
                with (
                    tc.tile_pool(name=f"pu{h}", bufs=1, space="PSUM") as pup,
                    tc.tile_pool(name=f"rstr{h}", bufs=3) as rsp,
                ):
                    u_ps = pup.tile([65, BN // 2], F32, tag="u")
                    for it in range(NT):
                        if it < NSBM:
                            mhalf = masks_sb[it][:, h * 4096:(h + 1) * 4096]
                        else:
                            mhalf_t = rsp.tile([128, 4096], BF16, tag="rst")
                            nc.sync.dma_start(
                                mhalf_t[:],
                                masks_dr[it - NSBM][:, h * 4096:(h + 1) * 4096])
                            mhalf = mhalf_t[:]
                        for jb in range(8):
                            nc.tensor.matmul(
                                u_ps[:, jb * 512:(jb + 1) * 512],
                                lhsT=m_aug[:, it * 65:(it + 1) * 65],
                                rhs=mhalf[:, jb * 512:(jb + 1) * 512],
                                start=(it == 0), stop=(it == NT - 1),
                                skip_group_check=True)
                    # u = 0.5*(u~ + S) applied on drain, cast fp16
                    for q in range(4):
                        ud = sp.tile([65, 1024], F16, tag="ud")
                        nc.vector.tensor_scalar(ud[:], u_ps[:, q * 1024:(q + 1) * 1024],
                                                s_col[:, 0:1], 0.5, ALU.add, ALU.mult)
                        nc.sync.dma_start(cc_in[h][:, q * 1024:(q + 1) * 1024],
                                          ud[:])
                nc.gpsimd.collective_compute(
                    "AllReduce", ALU.add, replica_groups=[list(range(NCORE))],
                    ins=[cc_in[h].opt()], outs=[cc_out[h].opt()])

            # ---- P3/P4: maskT via DMA XBAR transpose; v; z^T accumulation ----
            with (
                tc.tile_pool(name="roll", bufs=2) as rp,
                tc.tile_pool(name="pup2", bufs=3, space="PSUM") as pup2,
                tc.tile_pool(name="ptq", bufs=2, space="PSUM") as ptq,
                tc.tile_pool(name="pz", bufs=1, space="PSUM") as pzp,
            ):
                zt_ps = pzp.tile([C, N], F32, tag="zt")
                tq_ps = ptq.tile([1, C], F32, tag="tq")
                ur_sb = [pp.tile([65, BN // 2], F16, tag=f"ur{h}", name=f"ur{h}")
                         for h in range(2)]
                for h in range(2):
                    nc.gpsimd.dma_start(ur_sb[h][:], cc_out[h][:])
                for g in range(NGRP):
                    mtg = rp.tile([128, GJC * 1024], BF16, tag="mtg")
                    mtg3 = mtg[:].rearrange("p (a b) -> p a b", b=1024)
                    for it in range(NT):
                        if it < NSBM:
                            src = masks_sb[it][:, g * GJC * 128:(g + 1) * GJC * 128]
                        else:
                            src = masks_dr[it - NSBM][:, g * GJC * 128:(g + 1) * GJC * 128]
                        nc.sync.dma_start_transpose(
                            mtg3[:, :, it * 128:(it + 1) * 128], src)
                    for jl in range(GJC):
                        jc = g * GJC + jl
                        h = jc // 32
                        uch = ur_sb[h][:, (jc - h * 32) * 128:
                                       (jc - h * 32 + 1) * 128]
                        ut_ps = pup2.tile([128, 65], F16, tag="utp")
                        nc.tensor.transpose(ut_ps[:], uch, id16_sb[0:65, 0:65])
                        ut = sp.tile([128, 65], F32, tag="ut")
                        nc.scalar.copy(ut[:], ut_ps[:])
                        rec = sp.tile([128, 1], F32, tag="rec")
                        nc.vector.reciprocal(rec[:], ut[:, 64:65])
                        # v' = 0.5 * t / De  (bf16)
                        v = sp.tile([128, C], BF16, tag="v")
                        nc.vector.tensor_scalar(v[:], ut[:, 0:C], rec[:, 0:1],
                                                0.5, ALU.mult, ALU.mult)
                        for half in range(2):
                            nc.tensor.matmul(
                                zt_ps[:, half * 512:(half + 1) * 512],
                                lhsT=v[:],
                                rhs=mtg[:, jl * 1024 + half * 512:
                                        jl * 1024 + (half + 1) * 512],
                                start=(jc == 0), stop=False,
                                skip_group_check=True)
                        # T'[c] accumulation: sum_j v'[j, c] -> [1, C]
                        nc.tensor.matmul(tq_ps[:], lhsT=ones128[:, 0:1], rhs=v[:],
                                         start=(jc == 0), stop=(jc == JC - 1),
                                         skip_group_check=True)
                # rank-1 correction + local hyperedges into z^T
                tq16 = sp.tile([1, C], F16, tag="tq16")
                nc.scalar.copy(tq16[:], tq_ps[:])
                for half in range(2):
                    nc.tensor.matmul(zt_ps[:, half * 512:(half + 1) * 512],
                                     lhsT=tq16[:],
                                     rhs=ones512[:],
                                     start=False, stop=False,
                                     skip_group_check=True)
                for ec in range(2):
                    for half in range(2):
                        nc.tensor.matmul(zt_ps[:, half * 512:(half + 1) * 512],
                                         lhsT=vloc_sb[:, ec * C:(ec + 1) * C],
                                         rhs=hlt_sb[:, ec * N + half * 512:
                                                    ec * N + (half + 1) * 512],
                                         start=False,
                                         stop=(ec == 1),
                                         skip_group_check=True)
                # drain z^T, scale by dv2 along free dim (in place)
                nc.scalar.copy(zs_sb[:], zt_ps[:])
            nc.vector.tensor_tensor(zs_sb[:], zs_sb[:], dv2b[:], ALU.mult)

            # ---- P5: BN stats + AllReduce + coefficients ----
            ro = pp.tile([C, N], F32, tag="ro")
            stt = sp.tile([C, 2], F32, tag="stt")
            nc.vector.tensor_reduce(stt[:, 0:1], zs_sb[:],
                                    mybir.AxisListType.X, ALU.add)
            nc.vector.tensor_tensor(ro[:], zs_sb[:], zs_sb[:], ALU.mult)
            nc.vector.tensor_reduce(stt[:, 1:2], ro[:],
                                    mybir.AxisListType.X, ALU.add)
            st_in = dr.tile([C, 2], F32, tag="stin")
            st_out = dr.tile([C, 2], F32, tag="stout", addr_space="Shared")
            nc.sync.dma_start(st_in[:], stt[:])
            nc.gpsimd.collective_compute(
                "AllReduce", ALU.add, replica_groups=[list(range(NCORE))],
                ins=[st_in.opt()], outs=[st_out.opt()])
            stg = sp.tile([C, 2], F32, tag="stg")
            nc.sync.dma_start(stg[:], st_out[:])

            mu = sp.tile([C, 1], F32, tag="mu")
            nc.vector.tensor_scalar(mu[:], stg[:, 0:1], 1.0 / BN, None, ALU.mult)
            ex2 = sp.tile([C, 1], F32, tag="ex2")
            nc.vector.tensor_scalar(ex2[:], stg[:, 1:2], 1.0 / BN, None, ALU.mult)
            musq = sp.tile([C, 1], F32, tag="musq")
            nc.vector.tensor_tensor(musq[:], mu[:], mu[:], ALU.mult)
            var = sp.tile([C, 1], F32, tag="var")
            nc.vector.tensor_tensor(var[:], ex2[:], musq[:], ALU.subtract)
            eps_t = sp.tile([C, 1], F32, tag="eps")
            nc.vector.memset(eps_t[:], BN_EPS)
            sd = sp.tile([C, 1], F32, tag="sd")
            nc.scalar.activation(sd[:], var[:], AF.Sqrt, bias=eps_t[:, 0:1], scale=1.0)
            inv = sp.tile([C, 1], F32, tag="inv")
            nc.vector.reciprocal(inv[:], sd[:])
            sc_col = sp.tile([C, 1], F32, tag="sccol")
            nc.vector.tensor_tensor(sc_col[:], gb_sb[:, 0:1], inv[:], ALU.mult)
            ms = sp.tile([C, 1], F32, tag="ms")
            nc.vector.tensor_tensor(ms[:], mu[:], sc_col[:], ALU.mult)
            t_col = sp.tile([C, 1], F32, tag="tcol")
            nc.vector.tensor_tensor(t_col[:], gb_sb[:, 1:2], ms[:], ALU.subtract)

            # ---- P6: relu + residual + transpose out ----
            nc.scalar.activation(ro[:], zs_sb[:], AF.Relu,
                                 bias=t_col[:, 0:1], scale=sc_col[:, 0:1])
            nc.vector.tensor_tensor(ro[:], ro[:], xtr_sb[:], ALU.add)
            with tc.tile_pool(name="po", bufs=2, space="PSUM") as pop:
                for it in range(NT):
                    o_ps = pop.tile([128, C], F32, tag="ops")
                    nc.tensor.transpose(o_ps[:], ro[:, it * 128:(it + 1) * 128],
                                        id32_sb[0:C, 0:C])
                    ot = sp.tile([128, C], F32, tag="ot")
                    nc.scalar.copy(ot[:], o_ps[:])
                    nc.sync.dma_start(out[it * 128:(it + 1) * 128, :], ot[:])

    nc.compile()
    return nc


def _host_inputs(x, W_conv, b_conv, gamma, beta):
    bf = ml_dtypes.bfloat16
    xm = np.ascontiguousarray(x.reshape(BN, C).astype(np.float32))
    xT = np.ascontiguousarray(xm.T)                       # [64, 8192]
    hi32 = xT.astype(bf).astype(np.float32)
    lo32 = (xT - hi32).astype(bf).astype(np.float32)
    sq = (xm * xm).sum(1).astype(np.float32)
    sqh = sq.astype(bf).astype(np.float32)
    sql = (sq - sqh).astype(bf).astype(np.float32)
    sqll = (sq - sqh - sql).astype(bf).astype(np.float32)

    r1m = np.concatenate([2.0 * hi32, -sqh[None, :], -sql[None, :],
                          -sqll[None, :]], 0).astype(bf)   # [67, 8192]
    r2m = np.concatenate([2.0 * lo32, 2.0 * hi32], 0).astype(bf)  # [128, 8192]

    wbm = np.concatenate([W_conv.T.astype(np.float32),
                          b_conv[None, :].astype(np.float32)], 0).astype(bf)

    H_local = _local_incidence()
    cover = H_local.sum(1)
    dv2 = ((K + 1 + cover) ** -0.5).astype(np.float32)     # [1024]
    dv2t = dv2.reshape(NT, 128).T.copy()                   # [128, NT]

    hlocm = np.zeros((128, NT * E), np.float32)
    for it in range(NT):
        hlocm[:, it * E:(it + 1) * E] = H_local[it * 128:(it + 1) * 128, :]
    hltm = np.zeros((98, 2 * N), np.float32)
    for ec in range(2):
        hltm[:, ec * N:(ec + 1) * N] = H_local[:, ec * 98:(ec + 1) * 98].T

    ident = np.eye(128, dtype=np.float32)
    gbm = np.stack([gamma.astype(np.float32), beta.astype(np.float32)], 1)

    common = {
        "r1": np.ascontiguousarray(r1m),
        "r2": np.ascontiguousarray(r2m),
        "wbb": wbm,
        "dv2t": dv2t,
        "dv2r": dv2[None, :].copy(),
        "hloc": hlocm.astype(bf),
        "hlt": hltm.astype(bf),
        "id16": ident.astype(np.float16),
        "id32": ident,
        "gb": gbm,
    }
    in_maps = []
    for c in range(NCORE):
        m = dict(common)
        m["a1"] = np.ascontiguousarray(np.concatenate(
            [hi32[:, c * N:(c + 1) * N], np.ones((3, N), np.float32)], 0
        ).astype(bf))
        m["a2"] = np.ascontiguousarray(np.concatenate(
            [hi32[:, c * N:(c + 1) * N], lo32[:, c * N:(c + 1) * N]], 0
        ).astype(bf))
        m["xtr"] = np.ascontiguousarray(xT[:, c * N:(c + 1) * N])
        in_maps.append(m)
    return in_maps


def _get_nc():
    if "nc" not in _CACHE:
        _CACHE["nc"] = _build()
    return _CACHE["nc"]


def run_spmd(inputs, **kw):
    nc = _get_nc()
    in_maps = _host_inputs(inputs["x"], inputs["W_conv"], inputs["b_conv"],
                           inputs["gamma"], inputs["beta"])
    return bass_utils.run_bass_kernel_spmd(nc, in_maps, core_ids=list(range(NCORE)), **kw)


def kernel(**inputs):
    res = run_spmd(inputs)
    out = np.stack([res.results[c]["out"] for c in range(NCORE)], 0)
    return out.reshape(B, N, C).astype(np.float32)
